# revision 55
# baseline (speedup 1.0000x reference)
"""Trainium2 Bass kernel for nn_Attention_47717086658562 (sparse_attention).

Reference computation (B=256, S=1024, D=512):
    q = tgt @ W_q.T                              (B,1,D)
    r = src @ W_ref.T                            (B,S,D)
    u = v . tanh(q + r)                          (B,S)
    score = C * tanh(u), mask prev_idxs, softmax -> logits (B,1,S)
    attn_h = conv_w @ src^T + conv_b             (B,D,S)
    returns (attn_h, logits, mask_)

Strategy: data-parallel over batch, 32 batches per core on 8 NeuronCores.
Per batch the two big matmuls (r and attn_h) consume the same channel-major
source activations, so they are fused into a single stationary-weight
matrix [W_ref.T | conv_w.T] of shape (D, 2D) and computed in
(channel, seq) layout; src is pre-transposed to (B, D, S) bf16 on the host
during sharding so every device transfer is a plain contiguous DMA (the
DMA-xbar transpose path serializes against regular DMAs and stalls the
PE).  All matmuls are bf16 (fp32 runs at 1/4 rate on the PE).  The
v-weighted channel reduction of tanh(q+r) is folded into a per-batch
selector matmul that accumulates batch b's row into partition b of a
persistent PSUM tile, pipelined one batch behind the main stream; its two
512-column halves run concurrently in different PE column groups.  attn
stores ride the gpsimd SWDGE ring so they never head-of-line block the
HWDGE load ring.  Softmax over all 32 batches is one batched epilogue
(max-subtraction elided: scores are bounded by C=10).
"""

import numpy as np
import ml_dtypes

BZ, SRC, DIM = 256, 1024, 512
N_CORES = 8
C = 10.0
NEG = -1e30

_cache = {}


def _build(bpc):
    """Build + compile the per-core Bass graph for bpc batches per core."""
    import concourse.tile as tile
    import concourse.mybir as mybir
    from concourse import bacc

    fp32 = mybir.dt.float32
    bf16 = mybir.dt.bfloat16
    AF = mybir.ActivationFunctionType
    ALU = mybir.AluOpType

    KT = DIM // 128          # 4 contraction tiles
    MT = 2 * DIM // 128      # 8 output tiles (4 for r, 4 for attn)
    NH = SRC // 512          # 2 free-dim halves

    nc = bacc.Bacc("TRN2", target_bir_lowering=False, debug=False,
                   num_devices=N_CORES)

    src_e = nc.dram_tensor("src", [bpc, DIM, SRC], bf16, kind="ExternalInput").ap()
    wcat_e = nc.dram_tensor("wcat", [128, KT, 2 * DIM], bf16, kind="ExternalInput").ap()
    wq_e = nc.dram_tensor("wq", [128, KT, DIM], bf16, kind="ExternalInput").ap()
    tgt_e = nc.dram_tensor("tgtT", [128, KT, bpc], bf16, kind="ExternalInput").ap()
    v_e = nc.dram_tensor("vcol", [128, KT], fp32, kind="ExternalInput").ap()
    b_e = nc.dram_tensor("bcol", [128, KT], fp32, kind="ExternalInput").ap()
    sel_e = nc.dram_tensor("sel", [128, bpc, bpc], bf16, kind="ExternalInput").ap()
    nm_e = nc.dram_tensor("negmask", [bpc, SRC], fp32, kind="ExternalInput").ap()

    attn_e = nc.dram_tensor("attn", [bpc, DIM, SRC], bf16, kind="ExternalOutput").ap()
    log_e = nc.dram_tensor("logits", [bpc, SRC], fp32, kind="ExternalOutput").ap()

    with tile.TileContext(nc) as tc:
        with (
            tc.tile_pool(name="const", bufs=1) as cp,
            tc.tile_pool(name="srcp", bufs=4) as sp,
            tc.tile_pool(name="actp", bufs=2) as ap_,
            tc.tile_pool(name="attnp", bufs=6) as op_,
            tc.tile_pool(name="ps", bufs=5, space="PSUM") as pp,
            tc.tile_pool(name="psq", bufs=1, space="PSUM") as pq,
        ):
            # ---- constants (ordered so first-needed bytes land first) ----
            wq = cp.tile([128, KT, DIM], bf16, tag="wq")
            nc.sync.dma_start(out=wq[:], in_=wq_e[:])
            tgtT = cp.tile([128, KT, bpc], bf16, tag="tgtT")
            nc.sync.dma_start(out=tgtT[:], in_=tgt_e[:])
            # batch 0's first source chunk is issued ahead of wcat so the
            # main stream can start as soon as the q matmuls clear
            srcT0 = sp.tile([128, KT, SRC], bf16, tag="srcT")
            nc.sync.dma_start(out=srcT0[:, 0, :], in_=src_e[0, 0:128, :])
            wcat = cp.tile([128, KT, 2 * DIM], bf16, tag="wcat")
            nc.sync.dma_start(out=wcat[:, :, 0:DIM], in_=wcat_e[:, :, 0:DIM])
            for k in range(1, KT):
                nc.sync.dma_start(out=srcT0[:, k, :],
                                  in_=src_e[0, k * 128:(k + 1) * 128, :])
            vcol = cp.tile([128, KT], fp32, tag="vcol")
            bcol = cp.tile([128, KT], fp32, tag="bcol")
            sel = cp.tile([128, bpc, bpc], bf16, tag="sel")
            negmask = cp.tile([bpc, SRC], fp32, tag="negmask")

            q_sb = cp.tile([128, KT, bpc], fp32, tag="q_sb")
            # persistent PSUM accumulators for u: batch b's ones-reduction is
            # steered to partition b by a selector stationary matrix
            # one (2*bpc, 512) bank: half n lives at partitions [n*32, n*32+bpc)
            # so the two per-batch reduction matmuls run concurrently in
            # different PE column groups (tile_position)
            upsb = pq.tile([32 + bpc, 512], fp32, tag="upsb", bufs=1)
            ups = [upsb[0:bpc, :], upsb[32:32 + bpc, :]]

            # ---- prologue: q = tgt @ W_q.T, laid out (e, b) ----
            for et in range(KT):
                psq = pq.tile([128, bpc], fp32, tag="psq", bufs=2)
                for k in range(KT):
                    nc.tensor.matmul(
                        psq[:],
                        wq[:, k, et * 128:(et + 1) * 128],
                        tgtT[:, k, :],
                        start=(k == 0), stop=(k == KT - 1),
                    )
                nc.scalar.activation(q_sb[:, et, :], psq[:], AF.Copy)

            # ---- main loop over batches (u-reduction pipelined 1 behind) ----
            def sel_mm(bb, ww):
                for n in range(NH):
                    nc.tensor.matmul(
                        ups[n], sel[:, bb, :], ww[:, n * 512:(n + 1) * 512],
                        start=(bb == 0), stop=(bb == bpc - 1),
                        tile_position=(0, n * 32),
                    )

            w_prev = None
            for b in range(bpc):
                if b == 0:
                    srcT = srcT0
                else:
                    srcT = sp.tile([128, KT, SRC], bf16, tag="srcT")
                    for k in range(KT):
                        nc.sync.dma_start(
                            out=srcT[:, k, :],
                            in_=src_e[b, k * 128:(k + 1) * 128, :],
                        )
                if b == 0:
                    # deferred constants: not needed for the first few matmuls
                    nc.sync.dma_start(out=vcol[:], in_=v_e[:])
                    nc.sync.dma_start(out=bcol[:], in_=b_e[:])
                    nc.sync.dma_start(out=sel[:], in_=sel_e[:])
                    nc.sync.dma_start(out=wcat[:, :, DIM:2 * DIM],
                                      in_=wcat_e[:, :, DIM:2 * DIM])
                elif b == 1:
                    nc.sync.dma_start(out=negmask[:], in_=nm_e[:])

                tnh = ap_.tile([128, KT, SRC], bf16, tag="tnh")
                vt = ap_.tile([128, KT, SRC], bf16, tag="vt")
                w = ap_.tile([128, SRC], bf16, tag="w")

                for m in range(MT):
                    lo = m * 128
                    at = m - KT  # attn tile index if >= 0
                    if at >= 0:
                        asb = op_.tile([128, SRC], bf16, tag="attn")
                    for n in range(NH):
                        ps = pp.tile([128, 512], fp32, tag="ps")
                        for k in range(KT):
                            nc.tensor.matmul(
                                ps[:],
                                wcat[:, k, lo:lo + 128],
                                srcT[:, k, n * 512:(n + 1) * 512],
                                start=(k == 0), stop=(k == KT - 1),
                            )
                        if at >= 0:
                            # attn part: + conv_b, to bf16
                            nc.vector.tensor_scalar(
                                asb[:, n * 512:(n + 1) * 512], ps[:],
                                bcol[:, at:at + 1], None, ALU.add,
                            )
                        else:
                            # r part: tanh(q + r) -> bf16
                            nc.scalar.activation(
                                tnh[:, m, n * 512:(n + 1) * 512], ps[:],
                                AF.Tanh, bias=q_sb[:, m, b:b + 1],
                            )
                    if at >= 0:
                        # store on the SWDGE (gpsimd) ring so the HWDGE ring
                        # stays free for the next batch's loads
                        nc.gpsimd.dma_start(
                            out=attn_e[b, lo - DIM:lo - DIM + 128, :], in_=asb[:],
                        )
                    else:
                        # fold v in as soon as this channel-tile's tanh exists
                        nc.vector.tensor_scalar(
                            vt[:, m, :], tnh[:, m, :], vcol[:, m:m + 1], None,
                            ALU.mult,
                        )
                        if m == KT - 1:
                            a0 = ap_.tile([128, SRC], bf16, tag="a0")
                            a1 = ap_.tile([128, SRC], bf16, tag="a1")
                            nc.vector.tensor_tensor(a0[:], vt[:, 0, :], vt[:, 1, :], ALU.add)
                            nc.vector.tensor_tensor(a1[:], vt[:, 2, :], vt[:, 3, :], ALU.add)
                            nc.vector.tensor_tensor(w[:], a0[:], a1[:], ALU.add)
                    if m == 1 and b > 0:
                        # u-reduction of the previous batch, off the critical path
                        sel_mm(b - 1, w_prev)
                    if m == MT - 3 and b == bpc - 1:
                        # last batch: reduce early so the softmax epilogue
                        # overlaps the remaining attn matmuls
                        sel_mm(b, w)
                w_prev = w

            # ---- epilogue: batched masked softmax of C*tanh(u) ----
            # logits = softmax(C * (tanh(u) + negmask/C)) computed as
            # exp(C*s - C*max(s)) / sum
            # C*tanh(u) is bounded in [-10, 10] so exp never overflows in
            # fp32 and the usual max-subtraction can be skipped (softmax is
            # shift-invariant; the reference's max-sub changes nothing).
            # Ops are split per 512-half so ScalarE (tanh/exp) and VectorE
            # (mask-add) pipeline instead of serializing full-width.
            t_sb = cp.tile([bpc, SRC], fp32, tag="t_sb")
            s_sb = cp.tile([bpc, SRC], fp32, tag="s_sb")
            e_sb = cp.tile([bpc, SRC], fp32, tag="e_sb")
            rsum = cp.tile([bpc, NH], fp32, tag="rsum")
            for n in range(NH):
                h = slice(n * 512, (n + 1) * 512)
                nc.scalar.activation(t_sb[:, h], ups[n], AF.Tanh)
                nc.vector.tensor_tensor(s_sb[:, h], t_sb[:, h], negmask[:, h],
                                        ALU.add)
                nc.scalar.activation(e_sb[:, h], s_sb[:, h], AF.Exp,
                                     scale=C, accum_out=rsum[:, n:n + 1])
            rtot = cp.tile([bpc, 1], fp32, tag="rtot")
            nc.vector.tensor_tensor(rtot[:], rsum[:, 0:1], rsum[:, 1:2], ALU.add)
            rcp = cp.tile([bpc, 1], fp32, tag="rcp")
            nc.vector.reciprocal(rcp[:], rtot[:])
            lg = cp.tile([bpc, SRC], fp32, tag="lg")
            for n in range(NH):
                h = slice(n * 512, (n + 1) * 512)
                nc.vector.tensor_scalar(lg[:, h], e_sb[:, h], rcp[:], None,
                                        ALU.mult)
                # sync ring: idle at the end, and keeps the tail drain off
                # the slower SWDGE path
                nc.sync.dma_start(out=log_e[:, h], in_=lg[:, h])

    nc.compile()
    return nc


def _get_nc(bpc):
    if bpc not in _cache:
        _cache[bpc] = _build(bpc)
    return _cache[bpc]


def _prep_shared(W_q, W_ref, v, conv_w, conv_b):
    bf16 = ml_dtypes.bfloat16
    wcat = np.concatenate([W_ref.T, conv_w.T], axis=1)          # (D, 2D)
    wcat = np.ascontiguousarray(
        wcat.reshape(DIM // 128, 128, 2 * DIM).transpose(1, 0, 2)
    ).astype(bf16)
    wq = np.ascontiguousarray(
        W_q.T.reshape(DIM // 128, 128, DIM).transpose(1, 0, 2)
    ).astype(bf16)
    vcol = np.ascontiguousarray(v.reshape(DIM // 128, 128).T).astype(np.float32)
    bcol = np.ascontiguousarray(conv_b.reshape(DIM // 128, 128).T).astype(np.float32)
    return wcat, wq, vcol, bcol


def _make_sel(bpc):
    sel = np.zeros((128, bpc, bpc), dtype=ml_dtypes.bfloat16)
    for b in range(bpc):
        sel[:, b, b] = 1
    return sel


def kernel(src, tgt, mask, prev_idxs, W_q, W_ref, v, conv_w, conv_b):
    from concourse.bass_utils import run_bass_kernel_spmd

    bf16 = ml_dtypes.bfloat16
    src = np.asarray(src, dtype=np.float32)
    tgt = np.asarray(tgt, dtype=np.float32)
    mask = np.asarray(mask)
    prev_idxs = np.asarray(prev_idxs).astype(np.int64)
    W_q = np.asarray(W_q, dtype=np.float32)
    W_ref = np.asarray(W_ref, dtype=np.float32)
    v = np.asarray(v, dtype=np.float32)
    conv_w = np.asarray(conv_w, dtype=np.float32)
    conv_b = np.asarray(conv_b, dtype=np.float32)

    bz = src.shape[0]
    bpc = bz // N_CORES
    nc = _get_nc(bpc)

    wcat, wq, vcol, bcol = _prep_shared(W_q, W_ref, v, conv_w, conv_b)
    sel = _make_sel(bpc)
    # (B, S, D) -> (B, D, S) in bf16: the kernel consumes src channel-major
    src_b = np.ascontiguousarray(src.astype(bf16).transpose(0, 2, 1))

    # mask_ output (scatter of True at prev_idxs) on host; the device softmax
    # uses the equivalent additive -inf mask.
    mask_ = mask.copy()
    mask_[np.arange(bz), 0, prev_idxs] = True

    in_maps = []
    for i in range(N_CORES):
        sl = slice(i * bpc, (i + 1) * bpc)
        tgt_sh = tgt[sl, 0, :]                                   # (bpc, D)
        tgtT = np.ascontiguousarray(
            tgt_sh.T.reshape(DIM // 128, 128, bpc).transpose(1, 0, 2)
        ).astype(bf16)
        nm = np.where(mask_[sl, 0, :], np.float32(NEG / C), np.float32(0.0))
        in_maps.append({
            "src": src_b[sl],
            "wcat": wcat,
            "wq": wq,
            "tgtT": tgtT,
            "vcol": vcol,
            "bcol": bcol,
            "sel": sel,
            "negmask": np.ascontiguousarray(nm, dtype=np.float32),
        })

    res = run_bass_kernel_spmd(nc, in_maps, list(range(N_CORES)))

    attn = np.concatenate(
        [res.results[i]["attn"].astype(np.float32) for i in range(N_CORES)], axis=0
    )
    logits = np.concatenate(
        [res.results[i]["logits"] for i in range(N_CORES)], axis=0
    )[:, None, :]
    return attn, logits, mask_


# revision 58
# speedup vs baseline: 1.0057x; 1.0057x over previous
"""Trainium2 Bass kernel for nn_Attention_47717086658562 (sparse_attention).

Reference computation (B=256, S=1024, D=512):
    q = tgt @ W_q.T                              (B,1,D)
    r = src @ W_ref.T                            (B,S,D)
    u = v . tanh(q + r)                          (B,S)
    score = C * tanh(u), mask prev_idxs, softmax -> logits (B,1,S)
    attn_h = conv_w @ src^T + conv_b             (B,D,S)
    returns (attn_h, logits, mask_)

Strategy: data-parallel over batch, 32 batches per core on 8 NeuronCores.
Per batch the two big matmuls (r and attn_h) consume the same channel-major
source activations, so they are fused into a single stationary-weight
matrix [W_ref.T | conv_w.T] of shape (D, 2D) and computed in
(channel, seq) layout; src is pre-transposed to (B, D, S) bf16 on the host
during sharding so every device transfer is a plain contiguous DMA (the
DMA-xbar transpose path serializes against regular DMAs and stalls the
PE).  All matmuls are bf16 (fp32 runs at 1/4 rate on the PE).  The
v-weighted channel reduction of tanh(q+r) is folded into a per-batch
selector matmul that accumulates batch b's row into partition b of a
persistent PSUM tile, pipelined one batch behind the main stream; its two
512-column halves run concurrently in different PE column groups.  attn
stores ride the gpsimd SWDGE ring so they never head-of-line block the
HWDGE load ring.  Softmax over all 32 batches is one batched epilogue
(max-subtraction elided: scores are bounded by C=10).
"""

import numpy as np
import ml_dtypes

BZ, SRC, DIM = 256, 1024, 512
N_CORES = 8
C = 10.0
NEG = -1e30

_cache = {}


def _build(bpc):
    """Build + compile the per-core Bass graph for bpc batches per core."""
    import concourse.tile as tile
    import concourse.mybir as mybir
    from concourse import bacc

    fp32 = mybir.dt.float32
    bf16 = mybir.dt.bfloat16
    AF = mybir.ActivationFunctionType
    ALU = mybir.AluOpType

    KT = DIM // 128          # 4 contraction tiles
    MT = 2 * DIM // 128      # 8 output tiles (4 for r, 4 for attn)
    NH = SRC // 512          # 2 free-dim halves

    nc = bacc.Bacc("TRN2", target_bir_lowering=False, debug=False,
                   num_devices=N_CORES)

    src_e = nc.dram_tensor("src", [bpc, DIM, SRC], bf16, kind="ExternalInput").ap()
    wcat_e = nc.dram_tensor("wcat", [128, KT, 2 * DIM], bf16, kind="ExternalInput").ap()
    wq_e = nc.dram_tensor("wq", [128, KT, DIM], bf16, kind="ExternalInput").ap()
    tgt_e = nc.dram_tensor("tgtT", [128, KT, bpc], bf16, kind="ExternalInput").ap()
    v_e = nc.dram_tensor("vcol", [128, KT], fp32, kind="ExternalInput").ap()
    b_e = nc.dram_tensor("bcol", [128, KT], fp32, kind="ExternalInput").ap()
    sel_e = nc.dram_tensor("sel", [128, bpc, bpc], bf16, kind="ExternalInput").ap()
    nm_e = nc.dram_tensor("negmask", [bpc, SRC], fp32, kind="ExternalInput").ap()

    attn_e = nc.dram_tensor("attn", [bpc, DIM, SRC], bf16, kind="ExternalOutput").ap()
    log_e = nc.dram_tensor("logits", [bpc, SRC], fp32, kind="ExternalOutput").ap()

    with tile.TileContext(nc) as tc:
        with (
            tc.tile_pool(name="const", bufs=1) as cp,
            tc.tile_pool(name="srcp", bufs=4) as sp,
            tc.tile_pool(name="actp", bufs=2) as ap_,
            tc.tile_pool(name="attnp", bufs=6) as op_,
            tc.tile_pool(name="ps", bufs=5, space="PSUM") as pp,
            tc.tile_pool(name="psq", bufs=1, space="PSUM") as pq,
        ):
            # ---- constants (ordered so first-needed bytes land first) ----
            wq = cp.tile([128, KT, DIM], bf16, tag="wq")
            nc.sync.dma_start(out=wq[:], in_=wq_e[:])
            tgtT = cp.tile([128, KT, bpc], bf16, tag="tgtT")
            nc.sync.dma_start(out=tgtT[:], in_=tgt_e[:])
            wcat = cp.tile([128, KT, 2 * DIM], bf16, tag="wcat")
            nc.sync.dma_start(out=wcat[:, :, 0:DIM], in_=wcat_e[:, :, 0:DIM])
            vcol = cp.tile([128, KT], fp32, tag="vcol")
            bcol = cp.tile([128, KT], fp32, tag="bcol")
            sel = cp.tile([128, bpc, bpc], bf16, tag="sel")
            negmask = cp.tile([bpc, SRC], fp32, tag="negmask")

            q_sb = cp.tile([128, KT, bpc], fp32, tag="q_sb")
            # persistent PSUM accumulators for u: batch b's ones-reduction is
            # steered to partition b by a selector stationary matrix
            # one (2*bpc, 512) bank: half n lives at partitions [n*32, n*32+bpc)
            # so the two per-batch reduction matmuls run concurrently in
            # different PE column groups (tile_position)
            upsb = pq.tile([32 + bpc, 512], fp32, tag="upsb", bufs=1)
            ups = [upsb[0:bpc, :], upsb[32:32 + bpc, :]]

            # ---- HAM warm-up: the PE is idle waiting for weights during the
            # first ~4us anyway; dead-ended matmuls on (unwritten, garbage)
            # SBUF accumulate PE-busy time so the clock gate opens before the
            # real stream starts.  upsb is reset by sel_mm's start=True.
            junkA = cp.tile([128, 32 + bpc], bf16, tag="junkA")
            junkB = cp.tile([128, 512], bf16, tag="junkB")
            nc.vector.memset(junkA[:], 0.0)
            nc.vector.memset(junkB[:], 0.0)
            for _ in range(10):
                nc.tensor.matmul(upsb[:], junkA[:], junkB[:],
                                 start=True, stop=True)

            # ---- prologue: q = tgt @ W_q.T, laid out (e, b) ----
            for et in range(KT):
                psq = pq.tile([128, bpc], fp32, tag="psq", bufs=2)
                for k in range(KT):
                    nc.tensor.matmul(
                        psq[:],
                        wq[:, k, et * 128:(et + 1) * 128],
                        tgtT[:, k, :],
                        start=(k == 0), stop=(k == KT - 1),
                    )
                nc.scalar.activation(q_sb[:, et, :], psq[:], AF.Copy)

            # ---- main loop over batches (u-reduction pipelined 1 behind) ----
            def sel_mm(bb, ww):
                for n in range(NH):
                    nc.tensor.matmul(
                        ups[n], sel[:, bb, :], ww[:, n * 512:(n + 1) * 512],
                        start=(bb == 0), stop=(bb == bpc - 1),
                        tile_position=(0, n * 32),
                    )

            w_prev = None
            for b in range(bpc):
                srcT = sp.tile([128, KT, SRC], bf16, tag="srcT")
                for k in range(KT):
                    nc.sync.dma_start(
                        out=srcT[:, k, :],
                        in_=src_e[b, k * 128:(k + 1) * 128, :],
                    )
                if b == 0:
                    # deferred constants: not needed for the first few matmuls
                    nc.sync.dma_start(out=vcol[:], in_=v_e[:])
                    nc.sync.dma_start(out=bcol[:], in_=b_e[:])
                    nc.sync.dma_start(out=sel[:], in_=sel_e[:])
                    nc.sync.dma_start(out=wcat[:, :, DIM:2 * DIM],
                                      in_=wcat_e[:, :, DIM:2 * DIM])
                elif b == 1:
                    nc.sync.dma_start(out=negmask[:], in_=nm_e[:])

                tnh = ap_.tile([128, KT, SRC], bf16, tag="tnh")
                vt = ap_.tile([128, KT, SRC], bf16, tag="vt")
                w = ap_.tile([128, SRC], bf16, tag="w")

                for m in range(MT):
                    lo = m * 128
                    at = m - KT  # attn tile index if >= 0
                    if at >= 0:
                        asb = op_.tile([128, SRC], bf16, tag="attn")
                    for n in range(NH):
                        ps = pp.tile([128, 512], fp32, tag="ps")
                        for k in range(KT):
                            nc.tensor.matmul(
                                ps[:],
                                wcat[:, k, lo:lo + 128],
                                srcT[:, k, n * 512:(n + 1) * 512],
                                start=(k == 0), stop=(k == KT - 1),
                            )
                        if at >= 0:
                            # attn part: + conv_b, to bf16
                            nc.vector.tensor_scalar(
                                asb[:, n * 512:(n + 1) * 512], ps[:],
                                bcol[:, at:at + 1], None, ALU.add,
                            )
                        else:
                            # r part: tanh(q + r) -> bf16
                            nc.scalar.activation(
                                tnh[:, m, n * 512:(n + 1) * 512], ps[:],
                                AF.Tanh, bias=q_sb[:, m, b:b + 1],
                            )
                    if at >= 0:
                        # store on the SWDGE (gpsimd) ring so the HWDGE ring
                        # stays free for the next batch's loads
                        nc.gpsimd.dma_start(
                            out=attn_e[b, lo - DIM:lo - DIM + 128, :], in_=asb[:],
                        )
                    else:
                        # fold v in as soon as this channel-tile's tanh exists
                        nc.vector.tensor_scalar(
                            vt[:, m, :], tnh[:, m, :], vcol[:, m:m + 1], None,
                            ALU.mult,
                        )
                        if m == KT - 1:
                            a0 = ap_.tile([128, SRC], bf16, tag="a0")
                            a1 = ap_.tile([128, SRC], bf16, tag="a1")
                            nc.vector.tensor_tensor(a0[:], vt[:, 0, :], vt[:, 1, :], ALU.add)
                            nc.vector.tensor_tensor(a1[:], vt[:, 2, :], vt[:, 3, :], ALU.add)
                            nc.vector.tensor_tensor(w[:], a0[:], a1[:], ALU.add)
                    if m == 1 and b > 0:
                        # u-reduction of the previous batch, off the critical path
                        sel_mm(b - 1, w_prev)
                    if m == MT - 3 and b == bpc - 1:
                        # last batch: reduce early so the softmax epilogue
                        # overlaps the remaining attn matmuls
                        sel_mm(b, w)
                w_prev = w

            # ---- epilogue: batched masked softmax of C*tanh(u) ----
            # logits = softmax(C * (tanh(u) + negmask/C)) computed as
            # exp(C*s - C*max(s)) / sum
            # C*tanh(u) is bounded in [-10, 10] so exp never overflows in
            # fp32 and the usual max-subtraction can be skipped (softmax is
            # shift-invariant; the reference's max-sub changes nothing).
            # Ops are split per 512-half so ScalarE (tanh/exp) and VectorE
            # (mask-add) pipeline instead of serializing full-width.
            t_sb = cp.tile([bpc, SRC], fp32, tag="t_sb")
            s_sb = cp.tile([bpc, SRC], fp32, tag="s_sb")
            e_sb = cp.tile([bpc, SRC], fp32, tag="e_sb")
            rsum = cp.tile([bpc, NH], fp32, tag="rsum")
            for n in range(NH):
                h = slice(n * 512, (n + 1) * 512)
                nc.scalar.activation(t_sb[:, h], ups[n], AF.Tanh)
                nc.vector.tensor_tensor(s_sb[:, h], t_sb[:, h], negmask[:, h],
                                        ALU.add)
                nc.scalar.activation(e_sb[:, h], s_sb[:, h], AF.Exp,
                                     scale=C, accum_out=rsum[:, n:n + 1])
            rtot = cp.tile([bpc, 1], fp32, tag="rtot")
            nc.vector.tensor_tensor(rtot[:], rsum[:, 0:1], rsum[:, 1:2], ALU.add)
            rcp = cp.tile([bpc, 1], fp32, tag="rcp")
            nc.vector.reciprocal(rcp[:], rtot[:])
            lg = cp.tile([bpc, SRC], fp32, tag="lg")
            for n in range(NH):
                h = slice(n * 512, (n + 1) * 512)
                nc.vector.tensor_scalar(lg[:, h], e_sb[:, h], rcp[:], None,
                                        ALU.mult)
                # sync ring: idle at the end, and keeps the tail drain off
                # the slower SWDGE path
                nc.sync.dma_start(out=log_e[:, h], in_=lg[:, h])

    nc.compile()
    return nc


def _get_nc(bpc):
    if bpc not in _cache:
        _cache[bpc] = _build(bpc)
    return _cache[bpc]


def _prep_shared(W_q, W_ref, v, conv_w, conv_b):
    bf16 = ml_dtypes.bfloat16
    wcat = np.concatenate([W_ref.T, conv_w.T], axis=1)          # (D, 2D)
    wcat = np.ascontiguousarray(
        wcat.reshape(DIM // 128, 128, 2 * DIM).transpose(1, 0, 2)
    ).astype(bf16)
    wq = np.ascontiguousarray(
        W_q.T.reshape(DIM // 128, 128, DIM).transpose(1, 0, 2)
    ).astype(bf16)
    vcol = np.ascontiguousarray(v.reshape(DIM // 128, 128).T).astype(np.float32)
    bcol = np.ascontiguousarray(conv_b.reshape(DIM // 128, 128).T).astype(np.float32)
    return wcat, wq, vcol, bcol


def _make_sel(bpc):
    sel = np.zeros((128, bpc, bpc), dtype=ml_dtypes.bfloat16)
    for b in range(bpc):
        sel[:, b, b] = 1
    return sel


def kernel(src, tgt, mask, prev_idxs, W_q, W_ref, v, conv_w, conv_b):
    from concourse.bass_utils import run_bass_kernel_spmd

    bf16 = ml_dtypes.bfloat16
    src = np.asarray(src, dtype=np.float32)
    tgt = np.asarray(tgt, dtype=np.float32)
    mask = np.asarray(mask)
    prev_idxs = np.asarray(prev_idxs).astype(np.int64)
    W_q = np.asarray(W_q, dtype=np.float32)
    W_ref = np.asarray(W_ref, dtype=np.float32)
    v = np.asarray(v, dtype=np.float32)
    conv_w = np.asarray(conv_w, dtype=np.float32)
    conv_b = np.asarray(conv_b, dtype=np.float32)

    bz = src.shape[0]
    bpc = bz // N_CORES
    nc = _get_nc(bpc)

    wcat, wq, vcol, bcol = _prep_shared(W_q, W_ref, v, conv_w, conv_b)
    sel = _make_sel(bpc)
    # (B, S, D) -> (B, D, S) in bf16: the kernel consumes src channel-major
    src_b = np.ascontiguousarray(src.astype(bf16).transpose(0, 2, 1))

    # mask_ output (scatter of True at prev_idxs) on host; the device softmax
    # uses the equivalent additive -inf mask.
    mask_ = mask.copy()
    mask_[np.arange(bz), 0, prev_idxs] = True

    in_maps = []
    for i in range(N_CORES):
        sl = slice(i * bpc, (i + 1) * bpc)
        tgt_sh = tgt[sl, 0, :]                                   # (bpc, D)
        tgtT = np.ascontiguousarray(
            tgt_sh.T.reshape(DIM // 128, 128, bpc).transpose(1, 0, 2)
        ).astype(bf16)
        nm = np.where(mask_[sl, 0, :], np.float32(NEG / C), np.float32(0.0))
        in_maps.append({
            "src": src_b[sl],
            "wcat": wcat,
            "wq": wq,
            "tgtT": tgtT,
            "vcol": vcol,
            "bcol": bcol,
            "sel": sel,
            "negmask": np.ascontiguousarray(nm, dtype=np.float32),
        })

    res = run_bass_kernel_spmd(nc, in_maps, list(range(N_CORES)))

    attn = np.concatenate(
        [res.results[i]["attn"].astype(np.float32) for i in range(N_CORES)], axis=0
    )
    logits = np.concatenate(
        [res.results[i]["logits"] for i in range(N_CORES)], axis=0
    )[:, None, :]
    return attn, logits, mask_


# revision 59
# speedup vs baseline: 1.0095x; 1.0038x over previous
"""Trainium2 Bass kernel for nn_Attention_47717086658562 (sparse_attention).

Reference computation (B=256, S=1024, D=512):
    q = tgt @ W_q.T                              (B,1,D)
    r = src @ W_ref.T                            (B,S,D)
    u = v . tanh(q + r)                          (B,S)
    score = C * tanh(u), mask prev_idxs, softmax -> logits (B,1,S)
    attn_h = conv_w @ src^T + conv_b             (B,D,S)
    returns (attn_h, logits, mask_)

Strategy: data-parallel over batch, 32 batches per core on 8 NeuronCores.
Per batch the two big matmuls (r and attn_h) consume the same channel-major
source activations, so they are fused into a single stationary-weight
matrix [W_ref.T | conv_w.T] of shape (D, 2D) and computed in
(channel, seq) layout; src is pre-transposed to (B, D, S) bf16 on the host
during sharding so every device transfer is a plain contiguous DMA (the
DMA-xbar transpose path serializes against regular DMAs and stalls the
PE).  All matmuls are bf16 (fp32 runs at 1/4 rate on the PE).  The
v-weighted channel reduction of tanh(q+r) is folded into a per-batch
selector matmul that accumulates batch b's row into partition b of a
persistent PSUM tile, pipelined one batch behind the main stream; its two
512-column halves run concurrently in different PE column groups.  attn
stores ride the gpsimd SWDGE ring so they never head-of-line block the
HWDGE load ring.  Softmax over all 32 batches is one batched epilogue
(max-subtraction elided: scores are bounded by C=10).
"""

import numpy as np
import ml_dtypes

BZ, SRC, DIM = 256, 1024, 512
N_CORES = 8
C = 10.0
NEG = -1e30

_cache = {}


def _build(bpc):
    """Build + compile the per-core Bass graph for bpc batches per core."""
    import concourse.tile as tile
    import concourse.mybir as mybir
    from concourse import bacc

    fp32 = mybir.dt.float32
    bf16 = mybir.dt.bfloat16
    AF = mybir.ActivationFunctionType
    ALU = mybir.AluOpType

    KT = DIM // 128          # 4 contraction tiles
    MT = 2 * DIM // 128      # 8 output tiles (4 for r, 4 for attn)
    NH = SRC // 512          # 2 free-dim halves

    nc = bacc.Bacc("TRN2", target_bir_lowering=False, debug=False,
                   num_devices=N_CORES)

    src_e = nc.dram_tensor("src", [bpc, DIM, SRC], bf16, kind="ExternalInput").ap()
    wcat_e = nc.dram_tensor("wcat", [128, KT, 2 * DIM], bf16, kind="ExternalInput").ap()
    wq_e = nc.dram_tensor("wq", [128, KT, DIM], bf16, kind="ExternalInput").ap()
    tgt_e = nc.dram_tensor("tgtT", [128, KT, bpc], bf16, kind="ExternalInput").ap()
    v_e = nc.dram_tensor("vcol", [128, KT], fp32, kind="ExternalInput").ap()
    b_e = nc.dram_tensor("bcol", [128, KT], fp32, kind="ExternalInput").ap()
    sel_e = nc.dram_tensor("sel", [128, bpc, bpc], bf16, kind="ExternalInput").ap()
    nm_e = nc.dram_tensor("negmask", [bpc, SRC], fp32, kind="ExternalInput").ap()

    attn_e = nc.dram_tensor("attn", [bpc, DIM, SRC], bf16, kind="ExternalOutput").ap()
    log_e = nc.dram_tensor("logits", [bpc, SRC], fp32, kind="ExternalOutput").ap()

    with tile.TileContext(nc) as tc:
        with (
            tc.tile_pool(name="const", bufs=1) as cp,
            tc.tile_pool(name="srcp", bufs=4) as sp,
            tc.tile_pool(name="actp", bufs=2) as ap_,
            tc.tile_pool(name="attnp", bufs=6) as op_,
            tc.tile_pool(name="ps", bufs=5, space="PSUM") as pp,
            tc.tile_pool(name="psq", bufs=1, space="PSUM") as pq,
        ):
            # ---- constants (ordered so first-needed bytes land first) ----
            wq = cp.tile([128, KT, DIM], bf16, tag="wq")
            nc.sync.dma_start(out=wq[:], in_=wq_e[:])
            tgtT = cp.tile([128, KT, bpc], bf16, tag="tgtT")
            nc.sync.dma_start(out=tgtT[:], in_=tgt_e[:])
            wcat = cp.tile([128, KT, 2 * DIM], bf16, tag="wcat")
            nc.sync.dma_start(out=wcat[:, :, 0:DIM], in_=wcat_e[:, :, 0:DIM])
            vcol = cp.tile([128, KT], fp32, tag="vcol")
            bcol = cp.tile([128, KT], fp32, tag="bcol")
            sel = cp.tile([128, bpc, bpc], bf16, tag="sel")
            negmask = cp.tile([bpc, SRC], fp32, tag="negmask")

            q_sb = cp.tile([128, KT, bpc], fp32, tag="q_sb")
            # persistent PSUM accumulators for u: batch b's ones-reduction is
            # steered to partition b by a selector stationary matrix
            # one (2*bpc, 512) bank: half n lives at partitions [n*32, n*32+bpc)
            # so the two per-batch reduction matmuls run concurrently in
            # different PE column groups (tile_position)
            upsb = pq.tile([32 + bpc, 512], fp32, tag="upsb", bufs=1)
            ups = [upsb[0:bpc, :], upsb[32:32 + bpc, :]]

            # ---- HAM warm-up: the PE is idle waiting for weights during the
            # first ~4us anyway; dead-ended matmuls on (unwritten, garbage)
            # SBUF accumulate PE-busy time so the clock gate opens before the
            # real stream starts.  upsb is reset by sel_mm's start=True.
            junkA = cp.tile([128, 32 + bpc], bf16, tag="junkA")
            junkB = cp.tile([128, 512], bf16, tag="junkB")
            nc.vector.memset(junkA[:], 0.0)
            nc.vector.memset(junkB[:], 0.0)
            for _ in range(10):
                nc.tensor.matmul(upsb[:], junkA[:], junkB[:],
                                 start=True, stop=True)

            # ---- prologue: q = tgt @ W_q.T, laid out (e, b) ----
            for et in range(KT):
                psq = pq.tile([128, bpc], fp32, tag="psq", bufs=2)
                for k in range(KT):
                    nc.tensor.matmul(
                        psq[:],
                        wq[:, k, et * 128:(et + 1) * 128],
                        tgtT[:, k, :],
                        start=(k == 0), stop=(k == KT - 1),
                    )
                nc.scalar.activation(q_sb[:, et, :], psq[:], AF.Copy)

            # bridge the q->main DMA wait too, else HAM re-throttles
            for _ in range(6):
                nc.tensor.matmul(upsb[:], junkA[:], junkB[:],
                                 start=True, stop=True)

            # ---- main loop over batches (u-reduction pipelined 1 behind) ----
            def sel_mm(bb, ww):
                for n in range(NH):
                    nc.tensor.matmul(
                        ups[n], sel[:, bb, :], ww[:, n * 512:(n + 1) * 512],
                        start=(bb == 0), stop=(bb == bpc - 1),
                        tile_position=(0, n * 32),
                    )

            w_prev = None
            for b in range(bpc):
                srcT = sp.tile([128, KT, SRC], bf16, tag="srcT")
                for k in range(KT):
                    nc.sync.dma_start(
                        out=srcT[:, k, :],
                        in_=src_e[b, k * 128:(k + 1) * 128, :],
                    )
                if b == 0:
                    # deferred constants: not needed for the first few matmuls
                    nc.sync.dma_start(out=vcol[:], in_=v_e[:])
                    nc.sync.dma_start(out=bcol[:], in_=b_e[:])
                    nc.sync.dma_start(out=sel[:], in_=sel_e[:])
                    nc.sync.dma_start(out=wcat[:, :, DIM:2 * DIM],
                                      in_=wcat_e[:, :, DIM:2 * DIM])
                elif b == 1:
                    nc.sync.dma_start(out=negmask[:], in_=nm_e[:])

                tnh = ap_.tile([128, KT, SRC], bf16, tag="tnh")
                vt = ap_.tile([128, KT, SRC], bf16, tag="vt")
                w = ap_.tile([128, SRC], bf16, tag="w")

                for m in range(MT):
                    lo = m * 128
                    at = m - KT  # attn tile index if >= 0
                    if at >= 0:
                        asb = op_.tile([128, SRC], bf16, tag="attn")
                    for n in range(NH):
                        ps = pp.tile([128, 512], fp32, tag="ps")
                        for k in range(KT):
                            nc.tensor.matmul(
                                ps[:],
                                wcat[:, k, lo:lo + 128],
                                srcT[:, k, n * 512:(n + 1) * 512],
                                start=(k == 0), stop=(k == KT - 1),
                            )
                        if at >= 0:
                            # attn part: + conv_b, to bf16
                            nc.vector.tensor_scalar(
                                asb[:, n * 512:(n + 1) * 512], ps[:],
                                bcol[:, at:at + 1], None, ALU.add,
                            )
                        else:
                            # r part: tanh(q + r) -> bf16
                            nc.scalar.activation(
                                tnh[:, m, n * 512:(n + 1) * 512], ps[:],
                                AF.Tanh, bias=q_sb[:, m, b:b + 1],
                            )
                    if at >= 0:
                        # store on the SWDGE (gpsimd) ring so the HWDGE ring
                        # stays free for the next batch's loads
                        nc.gpsimd.dma_start(
                            out=attn_e[b, lo - DIM:lo - DIM + 128, :], in_=asb[:],
                        )
                    else:
                        # fold v in as soon as this channel-tile's tanh exists
                        nc.vector.tensor_scalar(
                            vt[:, m, :], tnh[:, m, :], vcol[:, m:m + 1], None,
                            ALU.mult,
                        )
                        if m == KT - 1:
                            a0 = ap_.tile([128, SRC], bf16, tag="a0")
                            a1 = ap_.tile([128, SRC], bf16, tag="a1")
                            nc.vector.tensor_tensor(a0[:], vt[:, 0, :], vt[:, 1, :], ALU.add)
                            nc.vector.tensor_tensor(a1[:], vt[:, 2, :], vt[:, 3, :], ALU.add)
                            nc.vector.tensor_tensor(w[:], a0[:], a1[:], ALU.add)
                    if m == 1 and b > 0:
                        # u-reduction of the previous batch, off the critical path
                        sel_mm(b - 1, w_prev)
                    if m == MT - 3 and b == bpc - 1:
                        # last batch: reduce early so the softmax epilogue
                        # overlaps the remaining attn matmuls
                        sel_mm(b, w)
                w_prev = w

            # ---- epilogue: batched masked softmax of C*tanh(u) ----
            # logits = softmax(C * (tanh(u) + negmask/C)) computed as
            # exp(C*s - C*max(s)) / sum
            # C*tanh(u) is bounded in [-10, 10] so exp never overflows in
            # fp32 and the usual max-subtraction can be skipped (softmax is
            # shift-invariant; the reference's max-sub changes nothing).
            # Ops are split per 512-half so ScalarE (tanh/exp) and VectorE
            # (mask-add) pipeline instead of serializing full-width.
            t_sb = cp.tile([bpc, SRC], fp32, tag="t_sb")
            s_sb = cp.tile([bpc, SRC], fp32, tag="s_sb")
            e_sb = cp.tile([bpc, SRC], fp32, tag="e_sb")
            rsum = cp.tile([bpc, NH], fp32, tag="rsum")
            for n in range(NH):
                h = slice(n * 512, (n + 1) * 512)
                nc.scalar.activation(t_sb[:, h], ups[n], AF.Tanh)
                nc.vector.tensor_tensor(s_sb[:, h], t_sb[:, h], negmask[:, h],
                                        ALU.add)
                nc.scalar.activation(e_sb[:, h], s_sb[:, h], AF.Exp,
                                     scale=C, accum_out=rsum[:, n:n + 1])
            rtot = cp.tile([bpc, 1], fp32, tag="rtot")
            nc.vector.tensor_tensor(rtot[:], rsum[:, 0:1], rsum[:, 1:2], ALU.add)
            rcp = cp.tile([bpc, 1], fp32, tag="rcp")
            nc.vector.reciprocal(rcp[:], rtot[:])
            lg = cp.tile([bpc, SRC], fp32, tag="lg")
            for n in range(NH):
                h = slice(n * 512, (n + 1) * 512)
                nc.vector.tensor_scalar(lg[:, h], e_sb[:, h], rcp[:], None,
                                        ALU.mult)
                # sync ring: idle at the end, and keeps the tail drain off
                # the slower SWDGE path
                nc.sync.dma_start(out=log_e[:, h], in_=lg[:, h])

    nc.compile()
    return nc


def _get_nc(bpc):
    if bpc not in _cache:
        _cache[bpc] = _build(bpc)
    return _cache[bpc]


def _prep_shared(W_q, W_ref, v, conv_w, conv_b):
    bf16 = ml_dtypes.bfloat16
    wcat = np.concatenate([W_ref.T, conv_w.T], axis=1)          # (D, 2D)
    wcat = np.ascontiguousarray(
        wcat.reshape(DIM // 128, 128, 2 * DIM).transpose(1, 0, 2)
    ).astype(bf16)
    wq = np.ascontiguousarray(
        W_q.T.reshape(DIM // 128, 128, DIM).transpose(1, 0, 2)
    ).astype(bf16)
    vcol = np.ascontiguousarray(v.reshape(DIM // 128, 128).T).astype(np.float32)
    bcol = np.ascontiguousarray(conv_b.reshape(DIM // 128, 128).T).astype(np.float32)
    return wcat, wq, vcol, bcol


def _make_sel(bpc):
    sel = np.zeros((128, bpc, bpc), dtype=ml_dtypes.bfloat16)
    for b in range(bpc):
        sel[:, b, b] = 1
    return sel


def kernel(src, tgt, mask, prev_idxs, W_q, W_ref, v, conv_w, conv_b):
    from concourse.bass_utils import run_bass_kernel_spmd

    bf16 = ml_dtypes.bfloat16
    src = np.asarray(src, dtype=np.float32)
    tgt = np.asarray(tgt, dtype=np.float32)
    mask = np.asarray(mask)
    prev_idxs = np.asarray(prev_idxs).astype(np.int64)
    W_q = np.asarray(W_q, dtype=np.float32)
    W_ref = np.asarray(W_ref, dtype=np.float32)
    v = np.asarray(v, dtype=np.float32)
    conv_w = np.asarray(conv_w, dtype=np.float32)
    conv_b = np.asarray(conv_b, dtype=np.float32)

    bz = src.shape[0]
    bpc = bz // N_CORES
    nc = _get_nc(bpc)

    wcat, wq, vcol, bcol = _prep_shared(W_q, W_ref, v, conv_w, conv_b)
    sel = _make_sel(bpc)
    # (B, S, D) -> (B, D, S) in bf16: the kernel consumes src channel-major
    src_b = np.ascontiguousarray(src.astype(bf16).transpose(0, 2, 1))

    # mask_ output (scatter of True at prev_idxs) on host; the device softmax
    # uses the equivalent additive -inf mask.
    mask_ = mask.copy()
    mask_[np.arange(bz), 0, prev_idxs] = True

    in_maps = []
    for i in range(N_CORES):
        sl = slice(i * bpc, (i + 1) * bpc)
        tgt_sh = tgt[sl, 0, :]                                   # (bpc, D)
        tgtT = np.ascontiguousarray(
            tgt_sh.T.reshape(DIM // 128, 128, bpc).transpose(1, 0, 2)
        ).astype(bf16)
        nm = np.where(mask_[sl, 0, :], np.float32(NEG / C), np.float32(0.0))
        in_maps.append({
            "src": src_b[sl],
            "wcat": wcat,
            "wq": wq,
            "tgtT": tgtT,
            "vcol": vcol,
            "bcol": bcol,
            "sel": sel,
            "negmask": np.ascontiguousarray(nm, dtype=np.float32),
        })

    res = run_bass_kernel_spmd(nc, in_maps, list(range(N_CORES)))

    attn = np.concatenate(
        [res.results[i]["attn"].astype(np.float32) for i in range(N_CORES)], axis=0
    )
    logits = np.concatenate(
        [res.results[i]["logits"] for i in range(N_CORES)], axis=0
    )[:, None, :]
    return attn, logits, mask_
